# revision 22
# baseline (speedup 1.0000x reference)
"""Trainium2 Bass kernel for single-head causal attention.

Problem: x[4, 4096, 100], Wq/Wk/Wv[100, 64] ->
         softmax(tril(x@Wq @ (x@Wk)^T / 8)) @ (x@Wv)   -> [4, 4096, 64]

Sharding (8 cores, SPMD single program):
  core = 2*b + c: batch b in 0..3, key-parity c in 0..1.
  Each core handles ALL 4096 queries of its batch but only the keys/values at
  global rows {c, c+2, c+4, ...} (2048 of them). This keeps the causal
  structure IDENTICAL across cores (local key tile kk is attended by global
  query columns q >= 256*kk, for both parities), so one program serves all 8
  cores; the one-column parity offset lives in a tiny [128, 256] data mask.
  Softmax is computed without max-subtraction (scores are bounded ~|s|<=9
  after the 1/8 scale, exp can't overflow), so the two half-key partials
  combine on the host as (num_A + num_B) / (den_A + den_B).

Per-core program (flash-attention style, scores kept transposed):
  qT = Wq^T x^T  [64, 4096],  kT = Wk^T xkv^T [64, 2048]  (bf16)
  V1[kk] = [x_kv@Wv | 1] per 128-key tile (bf16, ones col -> denominator)
  for each 512-query block qm (8 blocks):
    key tiles 0..2qm+1 in PAIRS (one exp call per pair, 3 rotating 2-bank
    PSUM buffers -- finer granularity starves the ACT engine less than
    bigger groups with fewer buffers):
      S^T[tile] = kT[tile]^T-block @ qT-block   (bf16 in, PSUM f32 [128,512])
      E = exp(S^T / 8)  (ACT engine, PSUM->SBUF bf16, one call per pair;
          the strip's last tile computes/exps only its valid 256 q-columns)
      boundary tiles: E *= mask (DVE)
      out' += V1[kk]^T @ E  (bf16 matmul, accumulates [65, 512] in PSUM,
          emitted TWO pairs late so the in-order PE streams ahead of exp)
    flush out' -> SBUF -> DRAM out[65, 4096]  (row 64 = sum exp = denom)

Perf notes (HW-traced; ~79us baseline -> ~60-63us):
  - PE HAM clock gate: the PE runs at 1.2 GHz until a ~3.4us fully-busy
    activity window passes, and the monitor watches datapath TOGGLES (all-
    zero data does not register -> iota fill). Fourteen dependency-free dummy
    matmuls bridge the DMA-latency head (whose landing jitters by ~2us) so
    the whole kernel runs at 2.4 GHz.
  - bf16 qT/kT (not fp32r): 128-col S^T stationary loads qualify for FWL
    (2x faster LDWEIGHTS) at an acceptable score-precision cost.
  - DMA: ~0.85us serial issue per dma_start and ~5us issue-to-semaphore
    latency. Inputs split across the sync+gpsimd queues, first-use chunks
    first; the scalar queue stays clear for the ~2.7us exp-table load.
  - qT/kT projections are dribbled 1-2 per pair behind the exps as PE
    filler; clumping them at strip boundaries starves ACT for ~2us each.
  - The last strip's flush copy rides the scalar queue (idle by then); its
    out-DMA rides the sync queue (DGE issue ~0.8us vs ~1.45us on scalar).
  - xkv0 rides the scalar HWDGE queue in parallel with sync's w3+xq0, and
    kT0 is projected before qT0 (its inputs land first): mid-kernel ACT
    gaps drop from ~4.2us to ~2.6us and the tail shortens, ~59.1us total.
  - fp8 DoubleRow AVs (tried, reverted): cutting PE work below ~90% busy
    makes the HAM clock gate re-throttle the PE mid-kernel and every
    matmul runs 1.5-2x slow; psA must stay at 3 bufs or the exp cadence
    leaks ~0.5us/group. See kernel_v158_fp8_wip in the transcript.
"""

import os
from contextlib import ExitStack

import numpy as np

B, T, E, H = 4, 4096, 100, 64
TK = T // 2  # keys per core
NKT = TK // 128  # 16 local key tiles
NQB = T // 512  # 8 query blocks
N_CORES = 8

_CACHE = {}


def _mask_np(c):
    """mask[i, j] = 1 if global key (2i+c) <= query col offset j else 0."""
    import ml_dtypes

    i = np.arange(128)[:, None]
    j = np.arange(256)[None, :]
    return (j >= 2 * i + c).astype(ml_dtypes.bfloat16)


def _build():
    if "nc" in _CACHE:
        return _CACHE["nc"]

    import concourse.bacc as bacc
    import concourse.tile as tile
    from concourse import mybir
    from concourse.bass import ts, ds

    f32 = mybir.dt.float32
    bf16 = mybir.dt.bfloat16
    Exp = mybir.ActivationFunctionType.Exp
    Mult = mybir.AluOpType.mult

    nc = bacc.Bacc("TRN2", target_bir_lowering=False, debug=False,
                   num_devices=N_CORES)

    xq_d = nc.dram_tensor("xq", [E, T], bf16, kind="ExternalInput").ap()
    xkv_d = nc.dram_tensor("xkv", [E, TK], bf16, kind="ExternalInput").ap()
    w3_d = nc.dram_tensor("w3", [E, 5 * H], bf16, kind="ExternalInput").ap()
    mask_d = nc.dram_tensor("mask", [128, 256], bf16,
                            kind="ExternalInput").ap()
    out_d = nc.dram_tensor("out", [H + 1, T], f32, kind="ExternalOutput").ap()

    with tile.TileContext(nc) as tc, ExitStack() as ctx:
        sb = ctx.enter_context(tc.tile_pool(name="sb", bufs=1))
        ep = ctx.enter_context(tc.tile_pool(name="ep", bufs=5))
        ob_p = ctx.enter_context(tc.tile_pool(name="ob", bufs=2))
        # PSUM budget (8 banks): tag "s" 3x[128,2,512] = 6 (shared by strip
        # pair-groups AND projection outputs), tag "o" 2x[128,512] = 2.
        psA = ctx.enter_context(tc.tile_pool(name="psA", bufs=3, space="PSUM"))
        ps_o = ctx.enter_context(tc.tile_pool(name="ps_o", bufs=2, space="PSUM"))

        xq_t = sb.tile([E, T], bf16)
        xkv_t = sb.tile([E, TK], bf16)
        # wq/wk stored with DUPLICATED columns ([wq|wq], [wk|wk]): the
        # qT/kT projections then need ONE M=128 matmul (FWL-eligible
        # 128-col stationary) instead of two col-group matmuls.
        w3_t = sb.tile([E, 5 * H], bf16)
        mask_t = sb.tile([128, 256], bf16)
        wqq_t = w3_t[:, 0:2 * H]
        wkk_t = w3_t[:, 2 * H:4 * H]
        wv_t = w3_t[:, 4 * H:5 * H]
        # qT/kT live duplicated in both partition halves (rows 0:64 ==
        # 64:128, via the duplicated-column weights) so S^T matmuls for
        # adjacent key tiles run CONCURRENTLY in the two halves of the
        # PE array. bf16 (not fp32r): the 128-col kT stationary loads then
        # qualify for FWL (2x faster LDWEIGHTS).
        qT_t = sb.tile([128, T], bf16)
        kT_t = sb.tile([128, TK], bf16)
        v1_t = sb.tile([128, NKT, H + 1], bf16)
        warm_t = sb.tile([128, 8], f32)
        wmm_t = sb.tile([128, 512], bf16)

        # DMA order = first-use order; few DMAs with LONG per-partition
        # lines (each dma_start costs ~0.85us of serial issue time on the
        # Sync queue, so batch aggressively).
        # Input DMAs spread across FOUR engine queues: a single queue moves
        # only ~130 GB/s, serializing the ~1.3MB of inputs until ~15us and
        # stalling the PE early (which re-throttles the HAM clock gate).
        # Each engine issues its DMAs before its own compute work needs it.
        # HAM warm-up: the PE clock gate only opens (1.2 -> 2.4 GHz) after a
        # fully-busy ~3.4us activity window, and the monitor watches datapath
        # TOGGLES — all-zero data doesn't register, hence iota fill. 14 dummy
        # matmuls of dependency-free PE work bridge the DMA head.
        nc.gpsimd.iota(wmm_t, [[1, 512]], channel_multiplier=1,
                       allow_small_or_imprecise_dtypes=True)
        wmm_ps = psA.tile([128, 512], f32, tag="s")
        for _ in range(14):
            nc.tensor.matmul(wmm_ps, wmm_t[:, 0:128], wmm_t,
                             start=True, stop=True)

        # The head chain is DMA-latency bound (~5us issue-to-semaphore for
        # ~100KB): land wm / xq0 / xkv0 in PARALLEL on three queues. The
        # scalar queue fits xkv0's issue before the ~2.7us exp-table load
        # (both finish well before the first real exp needs them).
        # All three critical head tensors ride the (deterministic) sync
        # queue back-to-back; gpsimd's SWDGE latency jitters by ~2us, so it
        # only carries chunks first used mid-kernel.
        nc.sync.dma_start(out=w3_t, in_=w3_d)
        nc.scalar.dma_start(out=xkv_t[:, 0:512], in_=xkv_d[:, 0:512])
        nc.sync.dma_start(out=xq_t[:, 0:512], in_=xq_d[:, 0:512])
        nc.gpsimd.dma_start(out=xkv_t[:, 512:1024], in_=xkv_d[:, 512:1024])
        nc.sync.dma_start(out=xq_t[:, 512:1024], in_=xq_d[:, 512:1024])
        nc.gpsimd.dma_start(out=xkv_t[:, 1024:2048], in_=xkv_d[:, 1024:2048])
        nc.sync.dma_start(out=xq_t[:, 1024:2048], in_=xq_d[:, 1024:2048])
        nc.gpsimd.dma_start(out=mask_t, in_=mask_d)
        nc.sync.dma_start(out=xq_t[:, 2048:3072], in_=xq_d[:, 2048:3072])
        nc.gpsimd.dma_start(out=xq_t[:, 3072:4096], in_=xq_d[:, 3072:4096])

        # First ACT instruction early: overlaps the ~2.7us exp-table load
        # with input DMA.
        nc.vector.memset(warm_t, 0.0)
        nc.scalar.activation(out=warm_t, in_=warm_t, func=Exp)
        nc.vector.memset(v1_t[:, :, H], 1.0)

        # ---- lazy projections, emitted just-in-time inside the main loop so
        # the PE has filler work while the ACT engine is the steady-state
        # bottleneck, and the first attention group starts early.
        qT_done = [False] * (T // 512)
        kT_done = [False] * (TK // 512)
        v_done = [False] * NKT

        def need_qT(j):
            if qT_done[j]:
                return
            qT_done[j] = True
            ps = psA.tile([128, 512], f32, tag="s")
            nc.tensor.matmul(ps, wqq_t, xq_t[:, ts(j, 512)],
                             start=True, stop=True)
            nc.vector.tensor_copy(qT_t[:, ts(j, 512)], ps)

        def need_kT(j):
            if kT_done[j]:
                return
            kT_done[j] = True
            ps = psA.tile([128, 512], f32, tag="s")
            nc.tensor.matmul(ps, wkk_t, xkv_t[:, ts(j, 512)],
                             start=True, stop=True)
            nc.vector.tensor_copy(kT_t[:, ts(j, 512)], ps)

        def need_v(kk):
            if v_done[kk]:
                return
            v_done[kk] = True
            ps = psA.tile([128, 512], f32, tag="s")
            nc.tensor.matmul(ps[:, :H], xkv_t[:, ts(kk, 128)], wv_t,
                             start=True, stop=True)
            nc.vector.tensor_copy(v1_t[:, kk, :H], ps[:, :H])

        def emit_needs(q):
            n = 2 * q + 2
            for j2 in range((n - 1) // 4 + 1):
                need_kT(j2)
            need_qT(q)

        # Projections for strip q+1 are queued at strip q's start and dribbled
        # out ONE per group (right after each exp is queued): clumping them at
        # a strip boundary stalls the next strip's S^T behind them on the
        # in-order PE, starving the ACT engine for ~2us per strip.
        from collections import deque
        proj_q = deque()

        # ---- main attention loop ----
        # Strip of key tiles per query block, processed in PSUM groups of up
        # to 3 tiles (one exp call per group). Each group's AV matmuls are
        # emitted one group LATE so the (in-order) PE runs the next group's
        # S^T matmuls while the ACT engine computes this group's exp.
        # S^T matmuls alternate PE row halves (tile kk%2), so adjacent ones
        # run CONCURRENTLY in the two halves of the PE array.
        def emit_av(p):
            e_t, tiles, o_t, nkk, qm, packed = p
            for i, kk in enumerate(tiles):
                pk = packed and kk == nkk - 1
                if pk:
                    nc.tensor.matmul(o_t[:H + 1, 256:512], v1_t[:, kk],
                                     e_t[:, i, 0:256],
                                     start=False, stop=True)
                else:
                    nc.tensor.matmul(o_t[:H + 1], v1_t[:, kk], e_t[:, i],
                                     start=(kk == 0),
                                     stop=(kk == nkk - 1))
            if tiles[-1] == nkk - 1:  # last group of qm: flush out'
                ob = ob_p.tile([H + 1, 512], f32, tag="ob")
                if qm == NQB - 1:
                    # tail chain on the scalar queue, idle after the last exp
                    nc.scalar.copy(ob, o_t[:H + 1])
                    nc.sync.dma_start(out=out_d[:, ds(512 * qm, 512)],
                                      in_=ob)
                else:
                    nc.vector.tensor_copy(ob, o_t[:H + 1])
                    nc.sync.dma_start(out=out_d[:, ds(512 * qm, 512)], in_=ob)

        pend = deque()
        for qm in range(NQB):
            nkk = 2 * qm + 2
            emit_needs(qm)  # normally a no-op (dribbled out earlier)
            if qm + 1 < NQB:
                nq = qm + 1
                proj_q.append(lambda q=nq: need_qT(q))
                jb = (2 * nq + 1) // 4
                if not kT_done[jb]:
                    proj_q.append(lambda j=jb: need_kT(j))
            groups = [[kk, kk + 1] for kk in range(0, nkk, 2)]
            if qm == NQB - 1:
                # shortest-possible final chain: last tile alone (256 cols)
                groups[-1:] = [[nkk - 2], [nkk - 1]]
            packed = True  # strip's last tile computes only 256 q-cols
            o_t = ps_o.tile([128, 512], f32, tag="o")
            qs_lo = qT_t[:H, ds(512 * qm, 512)]
            qs_hi = qT_t[H:128, ds(512 * qm, 512)]
            for gi, tiles in enumerate(groups):
                s_t = psA.tile([128, 2, 512], f32, tag="s")
                e_t = ep.tile([128, 2, 512], bf16, tag="e")
                cols = 0  # valid flattened cols in this group
                for i, kk in enumerate(tiles):
                    half = kk % 2
                    kts = kT_t[:H, ts(kk, 128)] if half == 0 \
                        else kT_t[H:128, ts(kk, 128)]
                    qs = qs_lo if half == 0 else qs_hi
                    if packed and kk == nkk - 1:
                        nc.tensor.matmul(s_t[:, i, 0:256], kts,
                                         qs[:, 256:512],
                                         start=True, stop=True)
                        cols += 256
                    else:
                        nc.tensor.matmul(s_t[:, i], kts, qs,
                                         start=True, stop=True)
                        cols += 512
                sf = s_t.rearrange("p a b -> p (a b)")
                ef = e_t.rearrange("p a b -> p (a b)")
                nc.scalar.activation(out=ef[:, :cols], in_=sf[:, :cols],
                                     func=Exp, scale=float(H) ** -0.5)
                if proj_q:
                    proj_q.popleft()()
                if qm >= 1 and proj_q:
                    # early strips are short: drain the projection queue at
                    # 2 units/group while all input chunks are now resident
                    proj_q.popleft()()
                for i, kk in enumerate(tiles):
                    # boundary masking: tile kk borders the causal diagonal
                    # at query cols [256*kk - 512*qm, +256)
                    lo = 256 * kk - 512 * qm
                    if 0 <= lo < 512:
                        co = 0 if (packed and kk == nkk - 1) else lo
                        nc.vector.tensor_tensor(e_t[:, i, co:co + 256],
                                                e_t[:, i, co:co + 256],
                                                mask_t, Mult)
                # V projections ride as PE filler; consumed one group later.
                for kk in tiles:
                    need_v(kk)
                if len(pend) >= 2:
                    emit_av(pend.popleft())
                pend.append((e_t, tiles, o_t, nkk, qm, packed))
        while pend:
            emit_av(pend.popleft())

    nc.compile()
    _CACHE["nc"] = nc
    return nc


def _bf16(a):
    import ml_dtypes

    return np.ascontiguousarray(a, dtype=np.float32).astype(ml_dtypes.bfloat16)


def _make_in_maps(x, Wq, Wk, Wv):
    import ml_dtypes

    x = np.asarray(x, dtype=np.float32)
    w3 = np.zeros((E, 5 * H), dtype=ml_dtypes.bfloat16)
    w3[:, 0:H] = _bf16(Wq)
    w3[:, H:2 * H] = _bf16(Wq)
    w3[:, 2 * H:3 * H] = _bf16(Wk)
    w3[:, 3 * H:4 * H] = _bf16(Wk)
    w3[:, 4 * H:5 * H] = _bf16(Wv)
    masks = [_mask_np(0), _mask_np(1)]
    in_maps = []
    for core in range(N_CORES):
        b, c = divmod(core, 2)
        in_maps.append({
            "xq": _bf16(x[b].T),
            "xkv": _bf16(x[b, c::2, :].T),
            "w3": w3,
            "mask": masks[c],
        })
    return in_maps


def _combine(results):
    out = np.empty((B, T, H), dtype=np.float32)
    for b in range(B):
        a = results[2 * b]["out"]
        bb = results[2 * b + 1]["out"]
        num = a[:H] + bb[:H]
        den = a[H] + bb[H]
        out[b] = (num / den).T
    return out


def run(x, Wq, Wk, Wv, trace=False):
    """Returns (output [4,4096,64] f32, exec_time_ns or None)."""
    from concourse.bass_utils import run_bass_kernel_spmd

    nc = _build()
    in_maps = _make_in_maps(x, Wq, Wk, Wv)
    res = run_bass_kernel_spmd(nc, in_maps, core_ids=list(range(N_CORES)),
                               trace=trace)
    return _combine(res.results), res


def kernel(x, Wq, Wk, Wv):
    out, _ = run(x, Wq, Wk, Wv, trace=False)
    return out

